# revision 4
# baseline (speedup 1.0000x reference)
"""Causal attention with RoPE, tensor-parallel over 8 NeuronCores. v3.

Problem: B=2, L=2048, d_model=2048, H=16 heads, D=128 head dim.
  qkv = X @ W_qkv  (per-head [q|k|v] column layout)
  Q,K rope'd (interleaved pairs), causal softmax(QK^T/sqrt(D)) @ V, @ W_out.

Sharding (Megatron-style): core c handles batch b=c//4 and head group
g=c%4 (4 heads). Each core computes a partial output; host sums 4
partials per batch.

Per-core dataflow:
  Phase A: X and W_qkv pre-quantized on host to fp8(e4m3). Corrected
    3-term DoubleRow matmuls run at 4x the fp32r PE rate:
      X@W ~= Xh@Wh + Xl@Wh + (Xh/32)@(32*Wl)
    The W-lo residual is pre-scaled by 32 on the host because it sits
    below e4m3's subnormal floor at its natural scale; the matching
    Xh/32 copy makes the product come out right (exponent shifts are
    exact). Q^T/K^T are produced directly in [d, l] layout (stationary
    = W tile) with head-dim permuted to [evens|odds]; V in [l, d].
    RoPE in [d, l]: U = raw*[cos|-sin] (DVE), W = raw*[sin|cos] (Pool),
    then pair-sum matmuls (P = [I64;I64]) fold the halves on the PE;
    results stored bf16.
  Phase B: per (head, 512-q-chunk): S^T = K^T.T @ Q^T in bf16, exp on
    ACT (scale 1/sqrt(D)) to bf16, 128-wide diagonal masks on DVE.
    Denominators via tiny ap=1 colsum matmuls on the PE (es stationary,
    bf16 ones moving, one accumulation group in a [128q, 4] PSUM bank),
    then reciprocal (DVE), PE transpose, selector-matmul broadcast.
    O^T = V.T @ E accumulated on PE; normalize on DVE -> fp8 hi/lo/his.
  Phase C: Y = O^T.T @ W_out as 3-term scaled fp8 DoubleRow over head
    pairs. Its matmuls are interleaved as PE filler between the next
    chunk's S blocks so the exp (ACT) pipeline never gates the PE.
"""
import math
import numpy as np
import ml_dtypes
import concourse.bacc as bacc
import concourse.mybir as mybir
import concourse.tile as tile
from concourse.bass_utils import run_bass_kernel_spmd

F32 = mybir.dt.float32
F32R = mybir.dt.float32r
BF16 = mybir.dt.bfloat16
F8 = mybir.dt.float8e4
AF = mybir.ActivationFunctionType
DR = mybir.MatmulPerfMode.DoubleRow

N_HEADS = 16
D = 128
THETA = 10000.0
B_FULL, L_FULL, DM_FULL = 2, 2048, 2048
H_PER_CORE = 4
N_CORES = 8
NP_F8 = ml_dtypes.float8_e4m3
NP_BF = ml_dtypes.bfloat16
WSC = 32.0   # W-lo residual pre-scale


def build_nc(L=L_FULL, DM=DM_FULL, H=H_PER_CORE):
    LT = L // 128           # l-tiles (16)
    KT = DM // 128          # contract tiles for projections (16)
    KP = KT // 2            # contract tile PAIRS for DoubleRow (8)
    HD = H * D              # qkv width per core (512)
    QC = L // 512           # q-chunks (4)
    OC = DM // 512          # out-proj n-chunks (4)
    ISQ = 1.0 / math.sqrt(D)

    nc = bacc.Bacc(None, target_bir_lowering=False)

    xd = {}
    for p in ("h", "l", "s"):   # hi, lo, hi/32
        xd[p] = nc.dram_tensor(f"x8{p}", [128, QC, KT, 512], F8,
                               kind="ExternalInput")
    w8_d = {}
    for w in ("wq", "wk"):
        for p in ("h", "l"):    # hi, 32*lo
            w8_d[w + p] = nc.dram_tensor(f"{w}8{p}", [128, H, KT, D], F8,
                                         kind="ExternalInput")
    for p in ("h", "l"):
        w8_d["wv" + p] = nc.dram_tensor(f"wv8{p}", [128, KT, HD], F8,
                                        kind="ExternalInput")
    cs1_d = nc.dram_tensor("cs1", [128, L], F32, kind="ExternalInput")
    cs2_d = nc.dram_tensor("cs2", [128, L], F32, kind="ExternalInput")
    pmat_d = nc.dram_tensor("pmat", [128, 64], F32R, kind="ExternalInput")
    mask_d = nc.dram_tensor("mask128", [128, 128], BF16, kind="ExternalInput")
    idm_d = nc.dram_tensor("idm", [128, 128], F32, kind="ExternalInput")
    sel_d = nc.dram_tensor("selmat", [4, 512], BF16, kind="ExternalInput")
    wo8h_d = nc.dram_tensor("wo8h", [128, H, DM], F8, kind="ExternalInput")
    wo8l_d = nc.dram_tensor("wo8l", [128, H, DM], F8, kind="ExternalInput")
    y_d = nc.dram_tensor("y", [L, DM], F32, kind="ExternalOutput")

    with tile.TileContext(nc) as tc:
        with tc.tile_pool(name="res", bufs=1) as resp:
            # resident across all phases
            qt_res = resp.tile([128, H, L], BF16)      # Q^T (d perm) 16KB
            kt_res = resp.tile([128, H, L], BF16)      # K^T (d perm) 16KB
            v4 = resp.tile([128, LT, HD], BF16)        # V [l,d]      16KB
            pmat = resp.tile([128, 64], F32R)
            mask = resp.tile([128, 128], BF16)
            idm = resp.tile([128, 128], F32)
            selm = resp.tile([4, 512], BF16)
            ones = resp.tile([128, 1], BF16)
            nc.vector.memset(ones[:], 1.0)

            # ---------------- Phase A ----------------
            with (
                tc.tile_pool(name="w8", bufs=1) as wp,
                tc.tile_pool(name="x8", bufs=2) as xp,
                tc.tile_pool(name="trig", bufs=1) as trp,
                tc.tile_pool(name="uw", bufs=2) as uwp,
                tc.tile_pool(name="psA", bufs=4, space="PSUM") as psA,
                tc.tile_pool(name="psR", bufs=2, space="PSUM") as psR,
            ):
                w8 = {}
                for k in w8_d:
                    shp = [128, KT, HD] if k.startswith("wv") else \
                        [128, H, KT, D]
                    w8[k] = wp.tile(shp, F8, tag=k, name=k)
                cs1 = trp.tile([128, L], F32)
                cs2 = trp.tile([128, L], F32)
                x_t = {}
                for lc in range(QC):
                    x_t[lc] = tuple(
                        xp.tile([128, KT, 512], F8, tag=f"x{p}",
                                name=f"x{p}{lc}") for p in ("h", "l", "s"))
                # DMA order = first-needed-first. wqh split per head so
                # the first matmul unit starts early.
                nc.sync.dma_start(w8["wqh"][:, 0], w8_d["wqh"][:, 0])
                for q in range(2):
                    nc.sync.dma_start(x_t[0][0][:, 8 * q:8 * (q + 1)],
                                      xd["h"][:, 0, 8 * q:8 * (q + 1)])
                for h in range(1, H):
                    nc.sync.dma_start(w8["wqh"][:, h], w8_d["wqh"][:, h])
                for q in range(2):
                    nc.sync.dma_start(x_t[0][1][:, 8 * q:8 * (q + 1)],
                                      xd["l"][:, 0, 8 * q:8 * (q + 1)])
                nc.sync.dma_start(x_t[0][2][:], xd["s"][:, 0])
                for h in range(H):
                    nc.sync.dma_start(w8["wql"][:, h], w8_d["wql"][:, h])
                nc.sync.dma_start(cs1[:, 0:512], cs1_d[:, 0:512])
                nc.sync.dma_start(cs2[:, 0:512], cs2_d[:, 0:512])
                nc.sync.dma_start(pmat[:], pmat_d[:, :])
                for h in range(H):
                    nc.sync.dma_start(w8["wkh"][:, h], w8_d["wkh"][:, h])
                nc.sync.dma_start(w8["wkl"][:], w8_d["wkl"][:, :])
                nc.sync.dma_start(w8["wvh"][:], w8_d["wvh"][:, :])
                nc.sync.dma_start(w8["wvl"][:], w8_d["wvl"][:, :])
                nc.sync.dma_start(cs1[:, 512:], cs1_d[:, 512:])
                nc.sync.dma_start(cs2[:, 512:], cs2_d[:, 512:])
                nc.sync.dma_start(mask[:], mask_d[:, :])
                nc.sync.dma_start(idm[:], idm_d[:, :])
                nc.sync.dma_start(selm[:], sel_d[:, :])

                # software pipeline: emit projection matmuls for one unit,
                # then the rope/copy tail of the previous unit
                pend = None   # (kind, h, lc, ps)

                def flush(nxt):
                    nonlocal pend
                    if pend is not None:
                        kind, h, lc, ps = pend
                        if kind == "v":
                            nc.scalar.activation(v4[:, lc, :], ps[:],
                                                 AF.Copy)
                        else:
                            dst = qt_res if kind == "q" else kt_res
                            u = uwp.tile([128, 512], F32R, tag="u")
                            w = uwp.tile([128, 512], F32R, tag="w")
                            sl = slice(512 * lc, 512 * (lc + 1))
                            nc.vector.tensor_mul(u[:], ps[:], cs1[:, sl])
                            nc.vector.tensor_mul(w[:], ps[:], cs2[:, sl])
                            pr = psR.tile([128, 512], F32, tag="pr")
                            nc.tensor.matmul(pr[0:64, :], pmat[:], u[:],
                                             start=True, stop=True)
                            nc.tensor.matmul(pr[64:128, :], pmat[:], w[:],
                                             start=True, stop=True)
                            nc.scalar.activation(dst[:, h, sl], pr[:],
                                                 AF.Copy)
                    pend = nxt

                for lc in range(QC):
                    xh, xl, xs = x_t[lc]
                    if lc > 0:
                        nc.sync.dma_start(xh[:], xd["h"][:, lc])
                        nc.sync.dma_start(xl[:], xd["l"][:, lc])
                        nc.sync.dma_start(xs[:], xd["s"][:, lc])
                    for wn, kind in (("wq", "q"), ("wk", "k")):
                        wh_t, wl_t = w8[wn + "h"], w8[wn + "l"]
                        terms = ((wh_t, xh), (wh_t, xl), (wl_t, xs))
                        if lc == 0 and wn == "wq":
                            # term-major: fill the PE while late DMAs land
                            qps = [psA.tile([128, 512], F32, tag="ps",
                                            name=f"q0ps{h}")
                                   for h in range(H)]
                            for t, (ww, xx) in enumerate(terms):
                                for h in range(H):
                                    for j in range(KP):
                                        jsl = slice(2 * j, 2 * j + 2)
                                        nc.tensor.matmul(
                                            qps[h][:], ww[:, h, jsl],
                                            xx[:, jsl],
                                            start=(t == 0 and j == 0),
                                            stop=(t == 2 and j == KP - 1),
                                            perf_mode=DR)
                            for h in range(H):
                                flush((kind, h, lc, qps[h]))
                            continue
                        for h in range(H):
                            ps = psA.tile([128, 512], F32, tag="ps")
                            for t, (ww, xx) in enumerate(terms):
                                for j in range(KP):
                                    jsl = slice(2 * j, 2 * j + 2)
                                    nc.tensor.matmul(
                                        ps[:], ww[:, h, jsl], xx[:, jsl],
                                        start=(t == 0 and j == 0),
                                        stop=(t == 2 and j == KP - 1),
                                        perf_mode=DR)
                            flush((kind, h, lc, ps))
                    # V for the 4 l-tiles of this chunk: stationary = X tile
                    wvh4, wvl4 = w8["wvh"], w8["wvl"]
                    for li in range(4):
                        ps = psA.tile([128, 512], F32, tag="ps")
                        lsl = slice(128 * li, 128 * (li + 1))
                        for t, (xx, ww) in enumerate(
                                ((xh, wvh4), (xl, wvh4), (xs, wvl4))):
                            for j in range(KP):
                                jsl = slice(2 * j, 2 * j + 2)
                                nc.tensor.matmul(
                                    ps[:], xx[:, jsl, lsl], ww[:, jsl],
                                    start=(t == 0 and j == 0),
                                    stop=(t == 2 and j == KP - 1),
                                    perf_mode=DR)
                        flush(("v", li, 4 * lc + li, ps))
                flush(None)

            # ---------------- Phase B + C ----------------
            with (
                tc.tile_pool(name="wo8", bufs=1) as wop,
                tc.tile_pool(name="es", bufs=3) as ep,
                tc.tile_pool(name="ot", bufs=2) as otp,
                tc.tile_pool(name="dn", bufs=2) as dnp,
                tc.tile_pool(name="yst", bufs=4) as yp,
                tc.tile_pool(name="psS", bufs=4, space="PSUM") as psS,
                tc.tile_pool(name="psO", bufs=1, space="PSUM") as psO,
                tc.tile_pool(name="psY", bufs=2, space="PSUM") as psY,
                tc.tile_pool(name="psd", bufs=1, space="PSUM") as psd,
            ):
                wo8h = wop.tile([128, H, DM], F8, tag="wo8h")
                wo8l = wop.tile([128, H, DM], F8, tag="wo8l")
                nc.sync.dma_start(wo8h[:], wo8h_d[:, :])
                nc.sync.dma_start(wo8l[:], wo8l_d[:, :])

                filler = []

                def pump(k):
                    n = 0
                    while filler and n < k:
                        try:
                            next(filler[0])
                            n += 1
                        except StopIteration:
                            filler.pop(0)

                def out_proj_gen(c, oh8, ol8, os8):
                    for li in range(4):
                        i = 4 * c + li
                        lsl = slice(128 * li, 128 * (li + 1))
                        ysb = yp.tile([128, OC, 512], F32, tag="ysb")
                        for o in range(OC):
                            osl = slice(512 * o, 512 * (o + 1))
                            psy = psY.tile([128, 512], F32, tag="psy")
                            for t, (oo, ww) in enumerate(
                                    ((oh8, wo8h), (ol8, wo8h), (os8, wo8l))):
                                for hp in range(H // 2):
                                    h2 = slice(2 * hp, 2 * hp + 2)
                                    nc.tensor.matmul(
                                        psy[:],
                                        oo[:, h2, lsl],
                                        ww[:, h2, osl],
                                        start=(t == 0 and hp == 0),
                                        stop=(t == 2 and hp == H // 2 - 1),
                                        perf_mode=DR)
                                    yield
                            nc.vector.tensor_copy(ysb[:, o, :], psy[:])
                            if c == QC - 1 and li == 3:
                                nc.sync.dma_start(
                                    y_d[128 * i:128 * (i + 1), osl],
                                    ysb[:, o, :])
                            elif o % 2 == 1:
                                nc.sync.dma_start(
                                    y_d[128 * i:128 * (i + 1),
                                        512 * (o - 1):512 * (o + 1)],
                                    ysb[:, o - 1:o + 1, :])
                            yield

                def den_tail_gen(pso, psden, h, oh8, ol8, os8):
                    # deferred den-chain: PE ops interleave with the NEXT
                    # unit's S blocks so the DVE reciprocal never gates PE
                    inv = dnp.tile([128, 4], F32, tag="inv")
                    nc.vector.reciprocal(inv[:], psden[:])
                    yield
                    pit = psd.tile([4, 128], F32, tag="dn", name="pit")
                    nc.tensor.transpose(pit[:], inv[:], idm[:])
                    invt = dnp.tile([4, 128], BF16, tag="invt")
                    nc.vector.tensor_copy(invt[:], pit[:])
                    yield
                    bc = psd.tile([128, 512], F32, tag="dn", name="bc")
                    for b in range(4):
                        bsl = slice(128 * b, 128 * (b + 1))
                        nc.tensor.matmul(bc[:, bsl], selm[:, bsl], invt[:],
                                         start=True, stop=True)
                    # DVE can read only one PSUM operand: stage bc in SBUF
                    bcs = dnp.tile([128, 512], BF16, tag="bcs")
                    nc.scalar.activation(bcs[:], bc[:], AF.Copy)
                    yield
                    ot = otp.tile([128, 512], F32R, tag="ot")
                    nc.vector.tensor_mul(ot[:], pso[:], bcs[:])
                    nc.vector.tensor_copy(oh8[:, h, :], ot[:])
                    yield
                    nc.vector.tensor_sub(ol8[:, h, :], ot[:], oh8[:, h, :])
                    nc.vector.tensor_scalar_mul(os8[:, h, :], ot[:],
                                                1.0 / WSC)
                    yield

                def attn_unit(h, c, oh8, ol8, os8, tail):
                    nblk = 4 * (c + 1)
                    es = ep.tile([128, LT, 512], BF16, tag="es")
                    # S blocks + exp + diagonal masks
                    for j in range(nblk):
                        m = j - 4 * c
                        qo = 128 * m if m > 0 else 0
                        pss = psS.tile([128, 512], F32, tag="pss")
                        nc.tensor.matmul(
                            pss[:, qo:],
                            kt_res[:, h, 128 * j:128 * (j + 1)],
                            qt_res[:, h, 512 * c + qo:512 * (c + 1)],
                            start=True, stop=True)
                        if tail is not None:
                            try:
                                next(tail)
                            except StopIteration:
                                tail = None
                        else:
                            pump(3)
                        nc.scalar.activation(es[:, j, qo:], pss[:, qo:],
                                             AF.Exp, scale=ISQ)
                        if m >= 0:
                            nc.vector.tensor_mul(
                                es[:, j, qo:qo + 128],
                                es[:, j, qo:qo + 128], mask[:])
                    if tail is not None:
                        for _ in tail:
                            pass
                    # O^T accumulation + denominator colsum matmuls
                    pso = psO.tile([128, 512], F32, tag="pso")
                    psden = psd.tile([128, 4], F32, tag="dn",
                                     name="psden")
                    for j in range(nblk):
                        m = j - 4 * c
                        qo = 128 * m if m > 0 else 0
                        nc.tensor.matmul(
                            pso[:, qo:],
                            v4[:, j, 128 * h:128 * (h + 1)],
                            es[:, j, qo:],
                            start=(j == 0), stop=(j == nblk - 1))
                        for b in range(max(0, m), 4):
                            # one accumulation group for the whole bank:
                            # zero-regions are 2KB-per-partition granular
                            nc.tensor.matmul(
                                psden[:, b:b + 1],
                                es[:, j, 128 * b:128 * (b + 1)],
                                ones[:],
                                start=(j == 0 and b == 0),
                                stop=(j == nblk - 1 and b == 3),
                                skip_group_check=True)
                        pump(1)
                    return den_tail_gen(pso, psden, h, oh8, ol8, os8)

                pending = None
                tail = None
                for c in range(QC):
                    oh8 = otp.tile([128, H, 512], F8, tag="oh8",
                                   name=f"oh8_{c}")
                    ol8 = otp.tile([128, H, 512], F8, tag="ol8",
                                   name=f"ol8_{c}")
                    os8 = otp.tile([128, H, 512], F8, tag="os8",
                                   name=f"os8_{c}")
                    for h in range(H):
                        tail = attn_unit(h, c, oh8, ol8, os8, tail)
                        if pending is not None and h == 0:
                            filler.append(out_proj_gen(*pending))
                            pending = None
                    pending = (c, oh8, ol8, os8)
                if tail is not None:
                    for _ in tail:
                        pass
                filler.append(out_proj_gen(*pending))
                pump(10 ** 9)

    nc.compile()
    return nc


# ---------------------------------------------------------------------------
# Host-side input prep


def _f8_trip(a):
    """hi, lo, hi/32 as fp8 (for the X side)."""
    hi = a.astype(NP_F8)
    lo = (a - hi.astype(np.float32)).astype(NP_F8)
    hs = (hi.astype(np.float32) / WSC).astype(NP_F8)
    return hi, lo, hs


def _w8_pair(a):
    """hi, 32*lo as fp8 (for the W side)."""
    hi = a.astype(NP_F8)
    lo = ((a - hi.astype(np.float32)) * WSC).astype(NP_F8)
    return hi, lo


def make_core_inputs(X, W_qkv, W_out, core, L=L_FULL, DM=DM_FULL,
                     H=H_PER_CORE):
    """Host-side sharding: core -> (batch, head-group) inputs."""
    KT = DM // 128
    QC = L // 512
    b = core // 4
    g = core % 4
    heads = list(range(g * H, (g + 1) * H))

    perm = np.concatenate([np.arange(0, D, 2), np.arange(1, D, 2)])
    w3 = W_qkv.reshape(DM, N_HEADS, 3 * D)
    wq = np.stack([w3[:, h, 0:D][:, perm] for h in heads], axis=1)   # DM H D
    wk = np.stack([w3[:, h, D:2 * D][:, perm] for h in heads], axis=1)
    wv = np.stack([w3[:, h, 2 * D:3 * D] for h in heads], axis=1)
    wo = W_out[g * H * D:(g + 1) * H * D, :]

    out = {}
    # X^T tiles: x8[p, c, j, t] = X[b][512c+t, 128j+p]
    xt = np.ascontiguousarray(X[b].T).astype(np.float32)   # [DM, L]
    xr = xt.reshape(KT, 128, QC, 512).transpose(1, 2, 0, 3)  # p c j t
    out["x8h"], out["x8l"], out["x8s"] = _f8_trip(np.ascontiguousarray(xr))
    for nm, w in (("wq", wq), ("wk", wk)):
        # [DM, H, D] -> [p, h, j, d]
        wr = w.astype(np.float32).reshape(KT, 128, H, D).transpose(1, 2, 0, 3)
        h8, l8 = _w8_pair(np.ascontiguousarray(wr))
        out[nm + "8h"], out[nm + "8l"] = h8, l8
    wvr = wv.reshape(DM, H * D).astype(np.float32)
    wvr = wvr.reshape(KT, 128, H * D).transpose(1, 0, 2)
    out["wv8h"], out["wv8l"] = _w8_pair(np.ascontiguousarray(wvr))
    wor = wo.astype(np.float32).reshape(H, 128, DM).transpose(1, 0, 2)
    out["wo8h"], out["wo8l"] = _w8_pair(np.ascontiguousarray(wor))

    inv_freq = 1.0 / (THETA ** (np.arange(0, D, 2, dtype=np.float32) / D))
    ang = np.arange(L, dtype=np.float32)[:, None] * inv_freq[None, :]
    cos = np.cos(ang).astype(np.float32).T    # [64, L]
    sin = np.sin(ang).astype(np.float32).T
    out["cs1"] = np.ascontiguousarray(np.concatenate([cos, -sin], axis=0))
    out["cs2"] = np.ascontiguousarray(np.concatenate([sin, cos], axis=0))

    pm = np.zeros((128, 64), dtype=np.float32)
    pm[np.arange(128), np.arange(128) % 64] = 1.0
    out["pmat"] = pm
    kk = np.arange(128)[:, None]
    tt = np.arange(128)[None, :]
    out["mask128"] = (tt >= kk).astype(NP_BF)
    out["idm"] = np.eye(128, dtype=np.float32)
    sel = np.zeros((4, 512), dtype=NP_BF)
    for bb in range(4):
        sel[bb, 128 * bb:128 * (bb + 1)] = 1.0
    out["selmat"] = sel
    return out


_NC_CACHE = {}


def get_nc():
    if "nc" not in _NC_CACHE:
        _NC_CACHE["nc"] = build_nc()
    return _NC_CACHE["nc"]


def kernel(X, W_qkv, W_out):
    X = np.asarray(X, dtype=np.float32)
    W_qkv = np.asarray(W_qkv, dtype=np.float32)
    W_out = np.asarray(W_out, dtype=np.float32)
    nc = get_nc()
    # cores c and c+4 share weight shards; trig/pmat/mask are global
    group_maps = [make_core_inputs(X, W_qkv, W_out, g) for g in range(4)]
    xb1 = make_core_inputs(X, W_qkv, W_out, 4)  # batch 1 x8
    in_maps = []
    for c in range(N_CORES):
        m = dict(group_maps[c % 4])
        if c >= 4:
            for k in ("x8h", "x8l", "x8s"):
                m[k] = xb1[k]
        in_maps.append(m)
    res = run_bass_kernel_spmd(nc, in_maps, list(range(N_CORES)))
    out = np.zeros((B_FULL, L_FULL, DM_FULL), dtype=np.float32)
    for c in range(N_CORES):
        out[c // 4] += res.results[c]["y"]
    return out
